# revision 11
# baseline (speedup 1.0000x reference)
"""Trainium2 Bass kernel for DualCrossMessageBlock (gnn message passing).

V2 design:
- Edges sharded by DESTINATION node range (core k owns nodes
  [1280k, 1280(k+1))); no collectives. Host pre-sorts edges by destination
  128-node block; segment-sum = selection-matrix matmul into PSUM per block.
- bf16 tables/messages. phi (MLP of s) is computed on device into a combined
  DRAM table [phi(768) | v(384)] bf16; ONE indirect gather per supertile of
  G*128 edges fetches both.
- Per-edge u-scalar algebra is folded into the TensorEngine: for each u
  component k, a scaled selection matrix S*u_k multiplies a window of
  [+-C | T] blocks, accumulating straight into the dv columns of the PSUM
  accumulator. VectorE only does the unavoidable elementwise products
  (x = phi_j*Wgate, x_vv*v_c, +-x_vc*v_d), G-batched to amortize op
  overheads. ScalarE evacuates Wgate PSUM->bf16 and scales half the S's.
"""

import sys

sys.path.insert(0, "/opt/trn_rl_repo")

import numpy as np
import ml_dtypes

BF16 = np.dtype(ml_dtypes.bfloat16)

N, E, F, R = 10000, 320000, 128, 20
NCORES = 8
NPAD = 10240  # 80 blocks of 128 nodes
BLOCKS_PER_CORE = 10
NODES_PER_CORE = BLOCKS_PER_CORE * 128  # 1280
G = 4  # subtiles (of 128 edges) per supertile; DVE ops batch across G
SINGLE_GATHER = True  # [128,G] multi-index gather wedges HW (mesh desync); use G singles

_CACHE = {}


def _build(t_b, n_pad=NPAD, blocks_per_core=BLOCKS_PER_CORE, ncores=NCORES):
    import concourse.bass as bass
    import concourse.bacc as bacc
    import concourse.tile as tile
    from concourse import mybir

    f32 = mybir.dt.float32
    f32r = mybir.dt.float32r
    bf16 = mybir.dt.bfloat16
    i32 = mybir.dt.int32
    MULT = mybir.AluOpType.mult
    ADD = mybir.AluOpType.add
    ISEQ = mybir.AluOpType.is_equal
    COPY = mybir.ActivationFunctionType.Copy

    assert t_b % G == 0
    npc = blocks_per_core * 128
    epc = blocks_per_core * t_b * 128
    n_st = blocks_per_core * (t_b // G)  # supertiles per core
    f6 = 6 * F
    ROW = f6 + 3 * F  # 1152 combined-table row

    nc = bacc.Bacc(
        "TRN2", target_bir_lowering=False, debug=False, num_devices=ncores
    )

    sT = nc.dram_tensor("sT", [F, n_pad], bf16, kind="ExternalInput").ap()
    vtab = nc.dram_tensor("vtab", [n_pad, 3 * F], bf16, kind="ExternalInput").ap()
    W1 = nc.dram_tensor("W1", [F, F], bf16, kind="ExternalInput").ap()
    b1 = nc.dram_tensor("b1", [F, 1], f32, kind="ExternalInput").ap()
    W2 = nc.dram_tensor("W2", [F, f6], bf16, kind="ExternalInput").ap()
    b2 = nc.dram_tensor("b2", [1, f6], bf16, kind="ExternalInput").ap()
    Wr = nc.dram_tensor("Wrcat", [2 * R + 1, f6], bf16, kind="ExternalInput").ap()
    rad = nc.dram_tensor("radcat", [2 * R + 1, epc], bf16, kind="ExternalInput").ap()
    ed4 = nc.dram_tensor("ed4", [n_st * 128, G * 16], f32,
                         kind="ExternalInput").ap()
    jof4 = nc.dram_tensor("jof4", [n_st * 128, G], i32, kind="ExternalInput").ap()
    svb = nc.dram_tensor("svbase", [npc, 4 * F], f32, kind="ExternalInput").ap()
    out = nc.dram_tensor("out", [npc, 4 * F], f32, kind="ExternalOutput").ap()

    with tile.TileContext(nc, num_cores=ncores) as tc:
        with (
            tc.tile_pool(name="dram", bufs=1, space="DRAM") as dpool,
            tc.tile_pool(name="const", bufs=1) as cpool,
        ):
            ctab = dpool.tile([n_pad, ROW], bf16)

            W1_s = cpool.tile([F, F], bf16)
            nc.sync.dma_start(out=W1_s[:], in_=W1[:, :])
            W2_s = cpool.tile([F, f6], bf16)
            nc.sync.dma_start(out=W2_s[:], in_=W2[:, :])
            b1_s = cpool.tile([F, 1], f32)
            nc.sync.dma_start(out=b1_s[:], in_=b1[:, :])
            b2_s = cpool.tile([1, f6], bf16)
            nc.sync.dma_start(out=b2_s[:], in_=b2[:, :])
            Wr_s = cpool.tile([2 * R + 1, f6], bf16)
            nc.sync.dma_start(out=Wr_s[:], in_=Wr[:, :])
            ones_s = cpool.tile([1, F], bf16)
            nc.vector.memset(ones_s[:], 1.0)
            iota_i = cpool.tile([128, 128], i32)
            nc.gpsimd.iota(
                iota_i[:], pattern=[[1, 128]], base=0, channel_multiplier=0
            )
            iota_b = cpool.tile([128, 128], bf16)
            nc.vector.tensor_copy(out=iota_b[:], in_=iota_i[:])

            # v half of the combined table
            nc.sync.dma_start(out=ctab[:, f6:ROW], in_=vtab[:, :])

            # ---- Phase A: phi table (MLP over all padded nodes) ----
            with (
                tc.tile_pool(name="phiw", bufs=3) as phiw,
                tc.tile_pool(name="phip", bufs=2, space="PSUM") as phip,
            ):
                for t in range(n_pad // F):
                    st_t = phiw.tile([F, F], bf16, tag="st")
                    nc.sync.dma_start(out=st_t[:], in_=sT[:, t * F:(t + 1) * F])
                    h_p = phip.tile([F, F], f32, tag="hp")
                    nc.tensor.matmul(
                        out=h_p[:], lhsT=W1_s[:], rhs=st_t[:], start=True, stop=True
                    )
                    sg_t = phiw.tile([F, F], f32, tag="sg")
                    nc.scalar.activation(
                        out=sg_t[:],
                        in_=h_p[:],
                        func=mybir.ActivationFunctionType.Sigmoid,
                        bias=b1_s[:, 0:1],
                        scale=1.0,
                    )
                    # silu(h+b1) = (h+b1) * sigmoid(h+b1)
                    hs_t = phiw.tile([F, F], bf16, tag="hs")
                    nc.vector.scalar_tensor_tensor(
                        out=hs_t[:], in0=h_p[:], scalar=b1_s[:, 0:1],
                        in1=sg_t[:], op0=ADD, op1=MULT,
                    )
                    for half in range(2):
                        ph_p = phip.tile([F, 384], f32, tag="php")
                        nc.tensor.matmul(
                            out=ph_p[:],
                            lhsT=hs_t[:],
                            rhs=W2_s[:, half * 384:(half + 1) * 384],
                            start=True,
                            stop=False,
                        )
                        nc.tensor.matmul(
                            out=ph_p[:],
                            lhsT=ones_s[:1, :],
                            rhs=b2_s[:1, half * 384:(half + 1) * 384],
                            start=False,
                            stop=True,
                        )
                        ph_s = phiw.tile([F, 384], bf16, tag="phs")
                        if half == 0:
                            nc.scalar.activation(out=ph_s[:], in_=ph_p[:], func=COPY)
                        else:
                            nc.vector.tensor_copy(out=ph_s[:], in_=ph_p[:])
                        nc.sync.dma_start(
                            out=ctab[t * F:(t + 1) * F, half * 384:(half + 1) * 384],
                            in_=ph_s[:],
                        )

            # ---- Phase B: edge supertiles ----
            with (
                tc.tile_pool(name="eio", bufs=3) as eio,
                tc.tile_pool(name="gat", bufs=2) as gat,
                tc.tile_pool(name="msg", bufs=2) as msg,
                tc.tile_pool(name="wgp", bufs=2, space="PSUM") as wgp,
                tc.tile_pool(name="accp", bufs=2, space="PSUM") as accp,
            ):
                st_per_block = t_b // G
                for blk in range(blocks_per_core):
                    acc = accp.tile([128, 4 * F], f32, tag="acc")
                    for sti in range(st_per_block):
                        st = blk * st_per_block + sti
                        e0 = st * G * 128
                        r0 = st * 128
                        rad_t = eio.tile([2 * R + 1, G * 128], bf16, tag="rad")
                        nc.sync.dma_start(out=rad_t[:], in_=rad[:, e0:e0 + G * 128])
                        ed_t = eio.tile([128, G * 16], f32, tag="ed")
                        nc.sync.dma_start(out=ed_t[:], in_=ed4[r0:r0 + 128, :])
                        jof_t = eio.tile([128, G], i32, tag="jof")
                        nc.sync.dma_start(out=jof_t[:], in_=jof4[r0:r0 + 128, :])

                        gt = gat.tile([128, G, ROW], bf16, tag="gt")
                        if SINGLE_GATHER:
                            for s in range(G):
                                nc.gpsimd.indirect_dma_start(
                                    out=gt[:, s, :],
                                    out_offset=None,
                                    in_=ctab[:, :],
                                    in_offset=bass.IndirectOffsetOnAxis(
                                        ap=jof_t[:, s:s + 1], axis=0
                                    ),
                                )
                        else:
                            nc.gpsimd.indirect_dma_start(
                                out=gt[:, :, :],
                                out_offset=None,
                                in_=ctab[:, :],
                                in_offset=bass.IndirectOffsetOnAxis(
                                    ap=jof_t[:, :], axis=0
                                ),
                            )

                        wgs = msg.tile([128, G, f6], bf16, tag="wgs")
                        for s in range(G):
                            wg_p0 = wgp.tile([128, 384], f32, tag="wgp0")
                            wg_p1 = wgp.tile([128, 384], f32, tag="wgp1")
                            nc.tensor.matmul(
                                out=wg_p0[:],
                                lhsT=rad_t[:, s * 128:(s + 1) * 128],
                                rhs=Wr_s[:, 0:384],
                                start=True, stop=True,
                            )
                            nc.tensor.matmul(
                                out=wg_p1[:],
                                lhsT=rad_t[:, s * 128:(s + 1) * 128],
                                rhs=Wr_s[:, 384:768],
                                start=True, stop=True,
                            )
                            nc.scalar.activation(
                                out=wgs[:, s, 0:384], in_=wg_p0[:], func=COPY
                            )
                            nc.scalar.activation(
                                out=wgs[:, s, 384:768], in_=wg_p1[:], func=COPY
                            )

                        M4 = msg.tile([128, G, 4 * F], bf16, tag="M4")
                        xg = msg.tile([128, G, 3 * F], bf16, tag="xg")
                        B11 = msg.tile([128, G, 5 * F], bf16, tag="B11")
                        B21 = msg.tile([128, G, 3 * F], bf16, tag="B21")
                        B12 = msg.tile([128, G, 5 * F], bf16, tag="B12")
                        B22 = msg.tile([128, G, 3 * F], bf16, tag="B22")

                        def gphi(c):
                            return gt[:, :, c * F:(c + 1) * F]

                        def gv(c):
                            return gt[:, :, f6 + c * F:f6 + (c + 1) * F]

                        def wchunk(c):
                            return wgs[:, :, c * F:(c + 1) * F]

                        TT = nc.vector.tensor_tensor
                        STT = nc.vector.scalar_tensor_tensor
                        # x chunks: s->M4[0], vv/vc1/vc2->xg, vs1/vs2->T slots
                        TT(out=M4[:, :, 0:F], in0=gphi(0), in1=wchunk(0), op=MULT)
                        TT(out=xg[:, :, 0:F], in0=gphi(1), in1=wchunk(1), op=MULT)
                        TT(out=B11[:, :, 2 * F:3 * F], in0=gphi(2), in1=wchunk(2),
                           op=MULT)
                        TT(out=B12[:, :, 2 * F:3 * F], in0=gphi(3), in1=wchunk(3),
                           op=MULT)
                        TT(out=xg[:, :, F:2 * F], in0=gphi(4), in1=wchunk(4),
                           op=MULT)
                        TT(out=xg[:, :, 2 * F:3 * F], in0=gphi(5), in1=wchunk(5),
                           op=MULT)
                        # T copies into second windows
                        nc.vector.tensor_copy(
                            out=B21[:, :, F:2 * F], in_=B11[:, :, 2 * F:3 * F]
                        )
                        nc.vector.tensor_copy(
                            out=B22[:, :, F:2 * F], in_=B12[:, :, 2 * F:3 * F]
                        )
                        # T0: M4 dv columns = x_vv * v_c
                        for c in range(3):
                            TT(out=M4[:, :, F + c * F:2 * F + c * F],
                               in0=xg[:, :, 0:F], in1=gv(c), op=MULT)
                        # +C and -C products into window slots
                        for (grp, B1x, B2x, xvc) in (
                            (0, B11, B21, xg[:, :, F:2 * F]),
                            (1, B12, B22, xg[:, :, 2 * F:3 * F]),
                        ):
                            # +C_0 -> B2[2], +C_1 -> B1[0], +C_2 -> B1[3]
                            TT(out=B2x[:, :, 2 * F:3 * F], in0=xvc, in1=gv(0),
                               op=MULT)
                            TT(out=B1x[:, :, 0:F], in0=xvc, in1=gv(1), op=MULT)
                            TT(out=B1x[:, :, 3 * F:4 * F], in0=xvc, in1=gv(2),
                               op=MULT)
                            # -C_0 -> B1[1], -C_1 -> B1[4], -C_2 -> B2[0]
                            STT(out=B1x[:, :, F:2 * F], in0=xvc, scalar=-1.0,
                                in1=gv(0), op0=MULT, op1=MULT)
                            STT(out=B1x[:, :, 4 * F:5 * F], in0=xvc, scalar=-1.0,
                                in1=gv(1), op0=MULT, op1=MULT)
                            STT(out=B2x[:, :, 0:F], in0=xvc, scalar=-1.0,
                                in1=gv(2), op0=MULT, op1=MULT)

                        # S family + matmuls, per subtile
                        S4 = msg.tile([128, G, 128], bf16, tag="S4")
                        SU1 = msg.tile([128, G, 384], bf16, tag="SU1")
                        SU2 = msg.tile([128, G, 384], bf16, tag="SU2")
                        for s in range(G):
                            iloc = ed_t[:, s * 16 + 12:s * 16 + 13]
                            nc.vector.tensor_scalar(
                                out=S4[:, s, :], in0=iota_b[:], scalar1=iloc,
                                scalar2=None, op0=ISEQ,
                            )
                            for k in range(3):
                                # S * u1k on DVE (fused iseq+mult), S * u2k on ACT
                                nc.vector.tensor_scalar(
                                    out=SU1[:, s, k * F:(k + 1) * F],
                                    in0=iota_b[:],
                                    scalar1=iloc,
                                    scalar2=ed_t[:, s * 16 + k:s * 16 + k + 1],
                                    op0=ISEQ, op1=MULT,
                                )
                                nc.scalar.activation(
                                    out=SU2[:, s, k * F:(k + 1) * F],
                                    in_=S4[:, s, :],
                                    func=COPY,
                                    scale=ed_t[:, s * 16 + 3 + k:s * 16 + 4 + k],
                                )
                        first = (sti == 0)
                        for s in range(G):
                            nc.tensor.matmul(
                                out=acc[:, :],
                                lhsT=S4[:, s, :],
                                rhs=M4[:, s, :],
                                start=(first and s == 0),
                                stop=False,
                            )
                            for (SUx, B1x, B2x) in ((SU1, B11, B21),
                                                    (SU2, B12, B22)):
                                # window k=0 -> B1[2F:5F], k=1 -> B2, k=2 -> B1[0:3F]
                                for k, rhs_ap in (
                                    (0, B1x[:, s, 2 * F:5 * F]),
                                    (1, B2x[:, s, 0:3 * F]),
                                    (2, B1x[:, s, 0:3 * F]),
                                ):
                                    last = (
                                        sti == st_per_block - 1 and s == G - 1
                                        and SUx is SU2 and k == 2
                                    )
                                    nc.tensor.matmul(
                                        out=acc[:, F:4 * F],
                                        lhsT=SUx[:, s, k * F:(k + 1) * F],
                                        rhs=rhs_ap,
                                        start=False,
                                        stop=last,
                                    )
                    svt = eio.tile([128, 4 * F], f32, tag="svt")
                    nc.sync.dma_start(
                        out=svt[:], in_=svb[blk * 128:(blk + 1) * 128, :]
                    )
                    ot = eio.tile([128, 4 * F], f32, tag="ot")
                    nc.vector.tensor_tensor(out=ot[:], in0=acc[:], in1=svt[:], op=ADD)
                    nc.sync.dma_start(
                        out=out[blk * 128:(blk + 1) * 128, :], in_=ot[:]
                    )

    nc.compile()
    return nc


def _prep(inputs, n_pad=NPAD, blocks_per_core=BLOCKS_PER_CORE, ncores=NCORES,
          n=N, f=F, r=R):
    """Host-side sharding/layout prep. Returns (t_b, in_maps)."""
    s = np.ascontiguousarray(np.asarray(inputs["s"], dtype=np.float32))
    v = np.ascontiguousarray(np.asarray(inputs["v"], dtype=np.float32))
    re1 = np.asarray(inputs["radial_embeddings_1"], dtype=np.float32)
    re2 = np.asarray(inputs["radial_embeddings_2"], dtype=np.float32)
    f1 = np.asarray(inputs["f_cut_1"], dtype=np.float32)
    f2 = np.asarray(inputs["f_cut_2"], dtype=np.float32)
    u1 = np.asarray(inputs["unit_vectors_1"], dtype=np.float32)
    u2 = np.asarray(inputs["unit_vectors_2"], dtype=np.float32)
    eidx = np.asarray(inputs["edge_index"]).astype(np.int64)
    W1 = np.asarray(inputs["W1"], dtype=np.float32)
    b1 = np.asarray(inputs["b1"], dtype=np.float32)
    W2 = np.asarray(inputs["W2"], dtype=np.float32)
    b2 = np.asarray(inputs["b2"], dtype=np.float32)
    Wrm = np.asarray(inputs["Wr"], dtype=np.float32)
    br = np.asarray(inputs["br"], dtype=np.float32)

    e = eidx.shape[1]
    i_idx, j_idx = eidx[0], eidx[1]
    nblocks = n_pad // 128

    g = i_idx // 128
    order = np.argsort(g, kind="stable")
    gs = g[order]
    counts = np.bincount(gs, minlength=nblocks)
    t_b = max(1, int(np.ceil(counts.max() / 128)))
    t_b = ((t_b + G - 1) // G) * G  # multiple of G for supertiling
    eblk = t_b * 128
    starts = np.concatenate([[0], np.cumsum(counts)[:-1]])
    pos = np.arange(e) - np.repeat(starts, counts)
    slot = gs * eblk + pos
    et = nblocks * eblk

    radcat = np.zeros((2 * r + 1, et), dtype=np.float32)
    edgedat = np.zeros((et, 16), dtype=np.float32)
    edgedat[:, 12] = 999.0  # pad edges match no node
    jfull = np.zeros(et, dtype=np.int32)

    io = i_idx[order]
    jo = j_idx[order]
    radcat[0:r, slot] = (re1 * f1[:, None])[order].T
    radcat[r:2 * r, slot] = (re2 * f2[:, None])[order].T
    radcat[2 * r, slot] = (f1 + f2)[order]
    edgedat[slot, 0:3] = u1[order]
    edgedat[slot, 3:6] = u2[order]
    edgedat[slot, 12] = (io % 128).astype(np.float32)
    jfull[slot] = jo.astype(np.int32)

    spad = np.zeros((n_pad, f), dtype=np.float32)
    spad[:n] = s
    vpad = np.zeros((n_pad, 3 * f), dtype=np.float32)
    vpad[:n] = v.reshape(n, 3 * f)
    svbase = np.concatenate([spad, vpad], axis=1)
    sT = np.ascontiguousarray(spad.T).astype(BF16)

    npc = blocks_per_core * 128
    epc = blocks_per_core * eblk
    n_st = epc // (G * 128)
    Wrcat = np.concatenate([Wrm, Wrm, br[None, :]], axis=0)

    # supertile-interleaved per-edge arrays: row st*128+p, sub-block k
    ed4 = np.ascontiguousarray(
        edgedat.reshape(-1, G, 128, 16).transpose(0, 2, 1, 3)
        .reshape(-1, G * 16))
    jof4 = np.ascontiguousarray(
        jfull.reshape(-1, G, 128).transpose(0, 2, 1).reshape(-1, G))

    in_maps = []
    for c in range(ncores):
        in_maps.append(dict(
            sT=sT,
            vtab=vpad.astype(BF16),
            W1=W1.astype(BF16),
            b1=np.ascontiguousarray(b1.reshape(f, 1)),
            W2=W2.astype(BF16),
            b2=np.ascontiguousarray(b2.reshape(1, 6 * f)).astype(BF16),
            Wrcat=Wrcat.astype(BF16),
            radcat=np.ascontiguousarray(radcat[:, c * epc:(c + 1) * epc]).astype(BF16),
            ed4=np.ascontiguousarray(ed4[c * n_st * 128:(c + 1) * n_st * 128]),
            jof4=np.ascontiguousarray(jof4[c * n_st * 128:(c + 1) * n_st * 128]),
            svbase=np.ascontiguousarray(svbase[c * npc:(c + 1) * npc]),
        ))
    return t_b, in_maps


def _make_runner(nc, ncores=NCORES):
    """Replicates bass2jax.run_bass_via_pjrt's multi-core path, but returns a
    reusable closure holding the jitted executable (so repeat runs don't
    re-trace) plus a bench hook for timing."""
    import jax
    import numpy as _np
    from jax.experimental.shard_map import shard_map
    from jax.sharding import Mesh, PartitionSpec
    from concourse import mybir
    from concourse.bass2jax import (
        _bass_exec_p,
        install_neuronx_cc_hook,
        partition_id_tensor,
    )

    install_neuronx_cc_hook()

    partition_name = (
        nc.partition_id_tensor.name if nc.partition_id_tensor else None
    )
    in_names, out_names, out_avals, zero_outs = [], [], [], []
    for alloc in nc.m.functions[0].allocations:
        if not isinstance(alloc, mybir.MemoryLocationSet):
            continue
        name = alloc.memorylocations[0].name
        if alloc.kind == "ExternalInput":
            if name != partition_name:
                in_names.append(name)
        elif alloc.kind == "ExternalOutput":
            shape = list(alloc.tensor_shape)
            npdt = _np.dtype(mybir.dt.np(alloc.dtype))
            out_names.append(name)
            out_avals.append(jax.core.ShapedArray(shape, npdt))
            zero_outs.append(_np.zeros(shape, npdt))

    n_params = len(in_names)
    n_outs = len(out_avals)
    in_names_all = list(in_names) + list(out_names)
    if partition_name is not None:
        in_names_all.append(partition_name)
    donate = tuple(range(n_params, n_params + n_outs))

    def _body(*args):
        operands = list(args)
        if partition_name is not None:
            operands.append(partition_id_tensor())
        outs = _bass_exec_p.bind(
            *operands,
            out_avals=tuple(out_avals),
            in_names=tuple(in_names_all),
            out_names=tuple(out_names),
            lowering_input_output_aliases=(),
            sim_require_finite=True,
            sim_require_nnan=True,
            nc=nc,
        )
        return tuple(outs)

    devices = jax.devices()[:ncores]
    mesh = Mesh(_np.asarray(devices), ("core",))
    in_specs = (PartitionSpec("core"),) * (n_params + n_outs)
    out_specs = (PartitionSpec("core"),) * n_outs
    sharded = jax.jit(
        shard_map(_body, mesh=mesh, in_specs=in_specs, out_specs=out_specs,
                  check_rep=False),
        donate_argnums=donate,
        keep_unused=True,
    )

    state = {}

    def run(in_maps):
        per_core = [[_np.asarray(m[name]) for name in in_names] for m in in_maps]
        concat_in = [
            _np.concatenate([per_core[c][i] for c in range(ncores)], axis=0)
            for i in range(n_params)
        ]
        state["concat_in"] = concat_in
        concat_zeros = [
            _np.zeros((ncores * z.shape[0], *z.shape[1:]), z.dtype)
            for z in zero_outs
        ]
        out_arrs = sharded(*concat_in, *concat_zeros)
        jax.block_until_ready(out_arrs)
        return [
            {
                name: _np.asarray(out_arrs[i]).reshape(
                    ncores, *out_avals[i].shape
                )[c]
                for i, name in enumerate(out_names)
            }
            for c in range(ncores)
        ]

    def bench(n=5):
        import time
        from jax.sharding import NamedSharding
        assert "concat_in" in state, "call run() first"
        shd = NamedSharding(mesh, PartitionSpec("core"))
        dev_in = [jax.device_put(x, shd) for x in state["concat_in"]]
        jax.block_until_ready(dev_in)
        times = []
        for _ in range(n):
            concat_zeros = [
                jax.device_put(
                    _np.zeros((ncores * z.shape[0], *z.shape[1:]), z.dtype), shd
                )
                for z in zero_outs
            ]
            jax.block_until_ready(concat_zeros)
            t0 = time.perf_counter()
            out_arrs = sharded(*dev_in, *concat_zeros)
            jax.block_until_ready(out_arrs)
            times.append(time.perf_counter() - t0)
        return times

    return run, bench


LAST_BENCH = None


def kernel(**inputs):
    global LAST_BENCH
    t_b, in_maps = _prep(inputs)
    if t_b not in _CACHE:
        nc = _build(t_b)
        _CACHE[t_b] = (nc,) + _make_runner(nc)
    nc, run, bench = _CACHE[t_b]
    LAST_BENCH = bench

    results = run(in_maps)
    full = np.concatenate([results[c]["out"] for c in range(NCORES)], axis=0)
    s_out = full[:N, :F]
    v_out = full[:N, F:].reshape(N, 3, F)
    return (s_out, v_out)
